# revision 8
# baseline (speedup 1.0000x reference)
"""Trainium2 Bass kernel for nn_CNNGenerator (frame CNN + FC + window-sum + FC).

Key algebraic facts exploited (validated vs the reference):
  * softmax over a size-1 axis == 1.0, so the whole attention_conv stack is
    dead code; the bmm reduces to an 8-wide sliding-window sum of ffc.
  * The per-window stride-2 conv stack collapses into global conv streams:
    an "interior" stream g{1,2,3} and a "left-edge" stream e{1,2,3} per
    layer, plus a 2-tap combine (z).  Per t:
      g1[s] = b1 + sum_k W1k x[s+k-8]          e1[t] = b1 + W11 x[t-7] + W12 x[t-6]
      g2[s] = b2 + V0 G1[s-2] + V1 G1[s] + V2 G1[s+2]
      e2[t] = b2 + V1 E1[t] + V2 G1[t+2]
      g3[s] = b3 + U0 G2[s-4] + U1 G2[s] + U2 G2[s+4]
      e3[t] = b3 + U1 E2[t] + U2 G2[t+4]
      z[t]  = b4 + T1 E3[t] + T2 G3[t+8]
    (capitals = leaky-activated streams), then fc1/fc2/fc3+tanh,
    ws[t] = sum_{d=-3..4} ffc[t+d], out = clip(fcw @ ws, 0, 1).

Sharding: pure data parallel, 2 batch elements per core on 8 cores.
On-chip layout: time axis split in 4 chunks of 2048; 32-channel streams pack
4 chunks x 32ch on the 128 partitions, 64-channel streams pack 2 chunks x 64ch
(two tiles).  Weights are host-packed into block-diagonal lhsT matrices.
"""
import sys

if '/opt/trn_rl_repo' not in sys.path:
    sys.path.insert(0, '/opt/trn_rl_repo')

import numpy as np
import ml_dtypes

BF16 = ml_dtypes.bfloat16

B, C, T = 16, 29, 8192
NCORES = 8
BPC = B // NCORES          # batch per core
Tc = T // 4                # time chunk
HL = 16                    # left halo: tile col u <-> global idx c*Tc + u - HL
W = Tc + 40                # per-batch stream tile width
W2 = BPC * W
NSLOT = 17                 # 128-col lhsT slots in the weight pack
NWCOL = NSLOT * 128 + 8    # + bias columns

_PROG = {}
PS_GROUP = 1024
PS_BUFS = 4
G1_DVE = False


def _leaky_np(x, s):
    return np.where(x >= 0, x, s * x)


def _blockdiag(blocks):
    k = sum(b.shape[0] for b in blocks)
    m = sum(b.shape[1] for b in blocks)
    out = np.zeros((k, m), np.float32)
    i = j = 0
    for b in blocks:
        out[i:i + b.shape[0], j:j + b.shape[1]] = b
        i += b.shape[0]
        j += b.shape[1]
    return out


def _pack_weights(inp):
    wp = np.zeros((128, NWCOL), np.float32)

    def put(slot, mat):
        wp[:mat.shape[0], slot * 128: slot * 128 + mat.shape[1]] = mat

    w1 = inp['w1'].astype(np.float32)  # [32, 29, 3]
    b1 = inp['b1'].astype(np.float32)
    # slots 0-2: g1 taps: blockdiag4 of [30, 32]: 29 in-ch rows + bias row
    for k in range(3):
        blk = np.zeros((30, 32), np.float32)
        blk[:29] = w1[:, :, k].T
        if k == 1:
            blk[29] = b1
        put(k, _blockdiag([blk] * 4))
    w2 = inp['w2'].astype(np.float32)
    for k in range(3):  # slots 3-5
        put(3 + k, _blockdiag([w2[:, :, k].T.astype(np.float32)] * 4))
    w3 = inp['w3'].astype(np.float32)
    for k in range(3):  # slots 6-8: [64, 128], duplicated at rows 64:128 so the
        # pair-1 matmuls (rhs base partition 64) see the same base
        blk = _blockdiag([w3[:, :, k].T.astype(np.float32)] * 2)
        put(6 + k, np.concatenate([blk, blk], axis=0))
    w4 = inp['w4'].astype(np.float32)
    for j in (1, 2):    # slots 9-10: [128, 128]
        put(9 + j - 1, _blockdiag([w4[:, :, j].T.astype(np.float32)] * 2))
    fw1t = inp['fw1'].T.astype(np.float32)
    put(11, np.concatenate([fw1t, fw1t], axis=0))               # [64,128] x2 rows
    # fc2 / fc3 as M=128 with zero column-halves: psum accumulation composes
    # the two chunk-halves onto partitions 0:64 / 64:128 without col-tiling.
    fw2t = inp['fw2'].T.astype(np.float32)          # [128, 64]
    z64 = np.zeros_like(fw2t)
    put(12, np.concatenate([fw2t, z64], axis=1))    # fc2_lo [128, 128]
    put(15, np.concatenate([z64, fw2t], axis=1))    # fc2_hi
    fw3t = _blockdiag([inp['fw3'].T.astype(np.float32)] * 2)       # [128, 64]
    z64b = np.zeros_like(fw3t)
    put(13, np.concatenate([fw3t, z64b], axis=1))   # fc3_lo
    put(16, np.concatenate([z64b, fw3t], axis=1))   # fc3_hi
    put(14, _blockdiag([inp['fcw'].T.astype(np.float32)] * 4))     # [128, 64]
    bc = NSLOT * 128
    wp[:, bc + 0] = np.tile(inp['b2'], 4)
    wp[:, bc + 1] = np.tile(inp['b3'], 2)
    wp[:, bc + 2] = np.tile(inp['b4'], 2)
    wp[:, bc + 3] = inp['fb1']
    wp[:, bc + 4] = np.tile(inp['fb2'], 2)
    wp[:, bc + 5] = np.tile(inp['fb3'], 4)
    return wp


def _split(lo, hi, step=512):
    return [(a, min(a + step, hi)) for a in range(lo, hi, step)]


def _build_program(reps=1):
    import concourse.bacc as bacc
    import concourse.mybir as mybir
    import concourse.tile as tile

    F32 = mybir.dt.float32
    F32R = mybir.dt.float32r
    BF = mybir.dt.bfloat16
    AF = mybir.ActivationFunctionType
    OP = mybir.AluOpType

    nc = bacc.Bacc("TRN2", target_bir_lowering=False, debug=False)
    x_d = nc.dram_tensor("x", [BPC, C + 1, T + 22], BF, kind="ExternalInput").ap()
    w_d = nc.dram_tensor("wpack", [128, NWCOL], BF, kind="ExternalInput").ap()
    o_d = nc.dram_tensor("out", [BPC, 16, T], BF, kind="ExternalOutput").ap()

    with tile.TileContext(nc) as tc:
        with tc.tile_pool(name="wp", bufs=1) as wpool, \
             tc.tile_pool(name="xp", bufs=1) as xpool, \
             tc.tile_pool(name="yp", bufs=1) as ypool, \
             tc.tile_pool(name="st", bufs=7) as spool, \
             tc.tile_pool(name="lk", bufs=2) as lkpool, \
             tc.tile_pool(name="ps", bufs=PS_BUFS, space="PSUM") as ppool:

            wsb = wpool.tile([128, NWCOL], BF, tag="w")
            nc.sync.dma_start(out=wsb[:], in_=w_d[:])
            bfs = wpool.tile([128, 8], F32, tag="bf")
            nc.scalar.activation(bfs[:, 0:6], wsb[:, NSLOT * 128: NSLOT * 128 + 6],
                                 mybir.ActivationFunctionType.Copy)

            def lhsT(slot, k=128, m=128, base=0):
                return wsb[base:base + k, slot * 128: slot * 128 + m]

            def bias(i):
                return bfs[:, i:i + 1]

            for _rep in range(reps):
                _emit_body(nc, tc, mybir, F32, F32R, BF, AF, OP, wsb, lhsT, bias,
                           xpool, ypool, spool, ppool, x_d, o_d, lkpool)
    nc.finalize()
    return nc


def _emit_body(nc, tc, mybir, F32, F32R, BF, AF, OP, wsb, lhsT, bias,
               xpool, ypool, spool, ppool, x_d, o_d, lkpool):
    if True:
        if True:
            # ---------------- input load: [120p = 4 x (29ch + ones), 2W]
            # host pre-pads x to [30, T+22] (10 zeros left, 12 right, ones row),
            # so each chunk load is one uniform DMA covering u in [6, Tc+28).
            X = xpool.tile([120, W2], BF, tag="x")
            NSEG = 4
            edges = [6 + (Tc + 22) * sg // NSEG for sg in range(NSEG + 1)]
            for sg in range(NSEG):
                lo, hi = edges[sg], edges[sg + 1]
                for b in range(BPC):
                    for c in range(4):
                        nc.sync.dma_start(
                            out=X[30 * c:30 * c + 30, b * W + lo: b * W + hi],
                            in_=x_d[b, :, c * Tc + lo - 6: c * Tc + hi - 6])

            ST = lambda nm: spool.tile([128, W2], BF, tag="st", name=nm)  # noqa: E731

            def conv_pass(out_tile, rng, groups, evac):
                """groups: list of (out_p0, out_p1, taps); taps: (lhsT_ap, rhs_tile,
                rp0, rp1, delta).  Matmuls fill a 4-bank psum group in 512-col
                bank slices; one evacuation op drains the whole group."""
                for b in range(BPC):
                    for (glo, ghi) in _split(rng[0], rng[1], PS_GROUP):
                        gn = ghi - glo
                        ps = ppool.tile([128, PS_GROUP], F32, tag="ps", name="ps")
                        for (p0, p1, taps) in groups:
                            for (lo, hi) in _split(glo, ghi, 512):
                                n, off = hi - lo, lo - glo
                                for i, (lw, rt, rp0, rp1, d) in enumerate(taps):
                                    tp = (lw.base_partition(), p0) if p0 else None
                                    nc.tensor.matmul(
                                        ps[p0:p1, off:off + n], lw,
                                        rt[rp0:rp1, b * W + lo + d: b * W + hi + d],
                                        start=(i == 0), stop=(i == len(taps) - 1),
                                        tile_position=tp)
                        evac(ps[:, 0:gn], out_tile[:, b * W + glo: b * W + ghi])

            def conv_pass_pair(passes, rng):
                """Interleave the matmuls of two 64-contraction streams so they
                land on PE row tiles (0,0)/(64,0) and overlap 2x.  passes:
                list of (out_tile, taps, evac); taps as in conv_pass but with
                rhs base partitions 0:64 / 64:128 respectively."""
                for b in range(BPC):
                    for (glo, ghi) in _split(rng[0], rng[1], PS_GROUP):
                        gn = ghi - glo
                        pss = [ppool.tile([128, PS_GROUP], F32, tag="ps",
                                          name="psp%d" % j)
                               for j in range(len(passes))]
                        for (lo, hi) in _split(glo, ghi, 512):
                            n, off = hi - lo, lo - glo
                            ntap = max(len(t) for (_, t, _) in passes)
                            for i in range(ntap):
                                for j, (ot, taps, evac) in enumerate(passes):
                                    if i >= len(taps):
                                        continue
                                    lw, rt, rp0, rp1, d = taps[i]
                                    nc.tensor.matmul(
                                        pss[j][0:128, off:off + n], lw,
                                        rt[rp0:rp1, b * W + lo + d: b * W + hi + d],
                                        start=(i == 0), stop=(i == len(taps) - 1),
                                        tile_position=(rp0, 0))
                        for j, (ot, taps, evac) in enumerate(passes):
                            evac(pss[j][:, 0:gn], ot[:, b * W + glo: b * W + ghi])

            def act_evac(func, bias_ap, alpha, rnd=True):
                def f(ps, ot):
                    nc.scalar.activation(ot, ps, func, bias=bias_ap, scale=1.0,
                                         alpha=alpha)
                return f

            def dve_leaky(alpha, bias_ap=None):
                def f(ps, ot):
                    lk = lkpool.tile([128, PS_GROUP], BF, tag="lk", name="lk")
                    n = ps.shape[-1]
                    if bias_ap is None:
                        nc.vector.tensor_scalar(lk[:, 0:n], ps, alpha, None, OP.mult)
                        nc.vector.tensor_tensor(ot, ps, lk[:, 0:n],
                                                OP.max)
                    else:
                        # leaky(ps + b): lk = alpha*(ps+b); out = max(ps+b, lk)
                        nc.vector.tensor_scalar(lk[:, 0:n], ps, bias_ap, alpha,
                                                OP.add, OP.mult)
                        nc.vector.scalar_tensor_tensor(
                            ot, ps, bias_ap, lk[:, 0:n],
                            OP.add, OP.max)
                return f

            G1 = ST("G1")
            conv_pass(G1, (14, Tc + 34),
                      [(0, 128, [(lhsT(k, 120), X, 0, 120, k - 8) for k in range(3)])],
                      act_evac(AF.Prelu, 0.0, 0.02))
            E1 = ST("E1")
            conv_pass(E1, (13, Tc + 21),
                      [(0, 128, [(lhsT(k, 120), X, 0, 120, k - 8) for k in (1, 2)])],
                      dve_leaky(0.02))
            G2 = ST("G2")
            conv_pass(G2, (17, Tc + 33),
                      [(0, 128, [(lhsT(3 + k), G1, 0, 128, 2 * (k - 1)) for k in range(3)])],
                      act_evac(AF.Prelu, bias(0), 0.02))
            E2 = ST("E2")
            conv_pass(E2, (13, Tc + 21),
                      [(0, 128, [(lhsT(4), E1, 0, 128, 0), (lhsT(5), G1, 0, 128, 2)])],
                      dve_leaky(0.02, bias(0)))
            G3 = [ST("G3a"), ST("G3b")]
            conv_pass_pair(
                [(G3[p], [(lhsT(6 + k, 64, base=64 * p), G2,
                           64 * p, 64 * p + 64, 4 * (k - 1)) for k in range(3)],
                  act_evac(AF.Prelu, bias(1), 0.2)) for p in range(2)],
                (21, Tc + 29))
            E3 = [ST("E3a"), ST("E3b")]
            conv_pass_pair(
                [(E3[p], [(lhsT(7, 64, base=64 * p), E2, 64 * p, 64 * p + 64, 0),
                          (lhsT(8, 64, base=64 * p), G2, 64 * p, 64 * p + 64, 4)],
                  dve_leaky(0.2, bias(1))) for p in range(2)],
                (13, Tc + 21))
            H = [ST("Ha"), ST("Hb")]
            for p in range(2):
                conv_pass(H[p], (13, Tc + 21),
                          [(0, 128, [(lhsT(9), E3[p], 0, 128, 0),
                                     (lhsT(10), G3[p], 0, 128, 8)])],
                          act_evac(AF.Prelu, bias(2), 0.2))
            H1 = [ST("H1" + str(cidx)) for cidx in range(4)]
            for p in range(2):
                conv_pass_pair(
                    [(H1[2 * p + half],
                      [(lhsT(11, 64, base=64 * half), H[p],
                        64 * half, 64 * half + 64, 0)],
                      act_evac(AF.Prelu, bias(3), 0.02)) for half in range(2)],
                    (13, Tc + 21))
            A2 = [ST("A2a"), ST("A2b")]
            for p in range(2):
                conv_pass(A2[p], (13, Tc + 21),
                          [(0, 128, [(lhsT(12), H1[2 * p], 0, 128, 0),
                                     (lhsT(15), H1[2 * p + 1], 0, 128, 0)])],
                          act_evac(AF.Prelu, bias(4), 0.02))
            FFC = ST("FFC")
            conv_pass(FFC, (13, Tc + 21),
                      [(0, 128, [(lhsT(13), A2[0], 0, 128, 0),
                                 (lhsT(16), A2[1], 0, 128, 0)])],
                      act_evac(AF.Tanh, bias(5), 0.0, rnd=False))

            # ffc outside the valid t-range [0, T) must read as ZERO in the
            # window sum (reference zero-pads ffc, not just x).  The streams
            # compute phantom values there (biases propagate through zero
            # input), so zero them: chunk0 cols t=-3..-1, chunk3 t=T..T+4.
            for b in range(BPC):
                nc.gpsimd.memset(FFC[0:32, b * W + 13: b * W + 16], 0.0)
                nc.gpsimd.memset(FFC[96:128, b * W + Tc + 16: b * W + Tc + 21], 0.0)

            # ---------------- window sum (DVE tree): ws[t] = sum_{d=-3..4} ffc[t+d]
            S1 = ST("S1")
            for b in range(BPC):
                o = b * W
                nc.vector.tensor_tensor(S1[:, o + 13: o + Tc + 19],
                                        FFC[:, o + 13: o + Tc + 19],
                                        FFC[:, o + 14: o + Tc + 20], OP.add)
            for b in range(BPC):
                o = b * W
                nc.vector.tensor_tensor(FFC[:, o + 13: o + Tc + 17],
                                        S1[:, o + 13: o + Tc + 17],
                                        S1[:, o + 15: o + Tc + 19], OP.add)
            for b in range(BPC):
                o = b * W
                nc.vector.tensor_tensor(S1[:, o + 16: o + Tc + 16],
                                        FFC[:, o + 13: o + Tc + 13],
                                        FFC[:, o + 17: o + Tc + 17], OP.add)

            # ---------------- final fc + clip -> Y [64 = 4ch x 16cls, BPC*Tc]
            Y = ypool.tile([64, BPC * Tc], BF, tag="y")
            for b in range(BPC):
                for (glo, ghi) in _split(16, Tc + 16, PS_GROUP):
                    ps = ppool.tile([128, PS_GROUP], F32, tag="ps", name="ps")
                    for (lo, hi) in _split(glo, ghi, 512):
                        n, off = hi - lo, lo - glo
                        nc.tensor.matmul(ps[0:64, off:off + n], lhsT(14, 128, 64),
                                         S1[:, b * W + lo: b * W + hi],
                                         start=True, stop=True)
                    nc.vector.tensor_scalar(
                        Y[:, b * Tc + glo - 16: b * Tc + ghi - 16],
                        ps[0:64, 0:ghi - glo], 0.0, 1.0, OP.max, OP.min)

            # ---------------- output DMA: Y rows 16c..16c+16 -> out[b, :, c*Tc:...]
            for b in range(BPC):
                for half in range(2):
                    h0, h1 = half * (Tc // 2), (half + 1) * (Tc // 2)
                    for c4 in range(4):
                        nc.sync.dma_start(
                            out=o_d[b, :, c4 * Tc + h0: c4 * Tc + h1],
                            in_=Y[16 * c4:16 * c4 + 16, b * Tc + h0: b * Tc + h1])


def _get_program(reps=1):
    global _PROG
    if _PROG is None:
        _PROG = {}
    if reps not in _PROG:
        _PROG[reps] = _build_program(reps)
    return _PROG[reps]


def kernel(**inputs):
    from concourse.bass_utils import run_bass_kernel_spmd

    x = np.asarray(inputs['speech_features'], np.float32)
    xa = np.zeros((B, C + 1, T + 22), np.float32)
    xa[:, :C, 10:10 + T] = x
    xa[:, C, :] = 1.0
    xa = xa.astype(BF16)
    wp = _pack_weights({k: np.asarray(v, np.float32) for k, v in inputs.items()
                        if k != 'speech_features'}).astype(BF16)
    nc = _get_program()
    in_maps = [{"x": xa[i * BPC:(i + 1) * BPC], "wpack": wp} for i in range(NCORES)]
    res = run_bass_kernel_spmd(nc, in_maps, core_ids=list(range(NCORES)))
    outs = [r["out"].transpose(0, 2, 1) for r in res.results]
    return np.ascontiguousarray(np.concatenate(outs, axis=0).astype(np.float32))



# revision 9
# speedup vs baseline: 1.1866x; 1.1866x over previous
"""Trainium2 Bass kernel for nn_CNNGenerator (frame CNN + FC + window-sum + FC).

Key algebraic facts exploited (validated vs the reference):
  * softmax over a size-1 axis == 1.0, so the whole attention_conv stack is
    dead code; the bmm reduces to an 8-wide sliding-window sum of ffc.
  * The per-window stride-2 conv stack collapses into global conv streams:
    an "interior" stream g{1,2,3} and a "left-edge" stream e{1,2,3} per
    layer, plus a 2-tap combine (z).  Per t:
      g1[s] = b1 + sum_k W1k x[s+k-8]          e1[t] = b1 + W11 x[t-7] + W12 x[t-6]
      g2[s] = b2 + V0 G1[s-2] + V1 G1[s] + V2 G1[s+2]
      e2[t] = b2 + V1 E1[t] + V2 G1[t+2]
      g3[s] = b3 + U0 G2[s-4] + U1 G2[s] + U2 G2[s+4]
      e3[t] = b3 + U1 E2[t] + U2 G2[t+4]
      z[t]  = b4 + T1 E3[t] + T2 G3[t+8]
    (capitals = leaky-activated streams), then fc1/fc2/fc3+tanh,
    ws[t] = sum_{d=-3..4} ffc[t+d], out = clip(fcw @ ws, 0, 1).

Sharding: pure data parallel, 2 batch elements per core on 8 cores.
On-chip layout: time axis split in 4 chunks of 2048; 32-channel streams pack
4 chunks x 32ch on the 128 partitions, 64-channel streams pack 2 chunks x 64ch
(two tiles).  Weights are host-packed into block-diagonal lhsT matrices.
"""
import sys

if '/opt/trn_rl_repo' not in sys.path:
    sys.path.insert(0, '/opt/trn_rl_repo')

import numpy as np
import ml_dtypes

BF16 = ml_dtypes.bfloat16

B, C, T = 16, 29, 8192
NCORES = 8
BPC = B // NCORES          # batch per core
Tc = T // 4                # time chunk
HL = 16                    # left halo: tile col u <-> global idx c*Tc + u - HL
W = Tc + 40                # per-batch stream tile width
W2 = BPC * W
NSLOT = 17                 # 128-col lhsT slots in the weight pack
NWCOL = NSLOT * 128 + 8    # + bias columns

_PROG = {}
PS_GROUP = 1024
PS_BUFS = 4
G1_DVE = False


def _leaky_np(x, s):
    return np.where(x >= 0, x, s * x)


def _blockdiag(blocks):
    k = sum(b.shape[0] for b in blocks)
    m = sum(b.shape[1] for b in blocks)
    out = np.zeros((k, m), np.float32)
    i = j = 0
    for b in blocks:
        out[i:i + b.shape[0], j:j + b.shape[1]] = b
        i += b.shape[0]
        j += b.shape[1]
    return out


def _pack_weights(inp):
    wp = np.zeros((128, NWCOL), np.float32)

    def put(slot, mat):
        wp[:mat.shape[0], slot * 128: slot * 128 + mat.shape[1]] = mat

    w1 = inp['w1'].astype(np.float32)  # [32, 29, 3]
    b1 = inp['b1'].astype(np.float32)
    # slots 0-2: g1 taps: blockdiag4 of [30, 32]: 29 in-ch rows + bias row
    for k in range(3):
        blk = np.zeros((30, 32), np.float32)
        blk[:29] = w1[:, :, k].T
        if k == 1:
            blk[29] = b1
        put(k, _blockdiag([blk] * 4))
    w2 = inp['w2'].astype(np.float32)
    for k in range(3):  # slots 3-5
        put(3 + k, _blockdiag([w2[:, :, k].T.astype(np.float32)] * 4))
    w3 = inp['w3'].astype(np.float32)
    for k in range(3):  # slots 6-8: [64, 128], duplicated at rows 64:128 so the
        # pair-1 matmuls (rhs base partition 64) see the same base
        blk = _blockdiag([w3[:, :, k].T.astype(np.float32)] * 2)
        put(6 + k, np.concatenate([blk, blk], axis=0))
    w4 = inp['w4'].astype(np.float32)
    for j in (1, 2):    # slots 9-10: [128, 128]
        put(9 + j - 1, _blockdiag([w4[:, :, j].T.astype(np.float32)] * 2))
    fw1t = inp['fw1'].T.astype(np.float32)
    put(11, np.concatenate([fw1t, fw1t], axis=0))               # [64,128] x2 rows
    # fc2 / fc3 as M=128 with zero column-halves: psum accumulation composes
    # the two chunk-halves onto partitions 0:64 / 64:128 without col-tiling.
    fw2t = inp['fw2'].T.astype(np.float32)          # [128, 64]
    z64 = np.zeros_like(fw2t)
    put(12, np.concatenate([fw2t, z64], axis=1))    # fc2_lo [128, 128]
    put(15, np.concatenate([z64, fw2t], axis=1))    # fc2_hi
    fw3t = _blockdiag([inp['fw3'].T.astype(np.float32)] * 2)       # [128, 64]
    z64b = np.zeros_like(fw3t)
    put(13, np.concatenate([fw3t, z64b], axis=1))   # fc3_lo
    put(16, np.concatenate([z64b, fw3t], axis=1))   # fc3_hi
    put(14, _blockdiag([inp['fcw'].T.astype(np.float32)] * 4))     # [128, 64]
    bc = NSLOT * 128
    wp[:, bc + 0] = np.tile(inp['b2'], 4)
    wp[:, bc + 1] = np.tile(inp['b3'], 2)
    wp[:, bc + 2] = np.tile(inp['b4'], 2)
    wp[:, bc + 3] = inp['fb1']
    wp[:, bc + 4] = np.tile(inp['fb2'], 2)
    wp[:, bc + 5] = np.tile(inp['fb3'], 4)
    return wp


def _split(lo, hi, step=512):
    return [(a, min(a + step, hi)) for a in range(lo, hi, step)]


def _build_program(reps=1):
    import concourse.bacc as bacc
    import concourse.mybir as mybir
    import concourse.tile as tile

    F32 = mybir.dt.float32
    F32R = mybir.dt.float32r
    BF = mybir.dt.bfloat16
    AF = mybir.ActivationFunctionType
    OP = mybir.AluOpType

    nc = bacc.Bacc("TRN2", target_bir_lowering=False, debug=False)
    x_d = nc.dram_tensor("x", [BPC, 4, C + 1, Tc + 22], BF, kind="ExternalInput").ap()
    w_d = nc.dram_tensor("wpack", [128, NWCOL], BF, kind="ExternalInput").ap()
    o_d = nc.dram_tensor("out", [BPC, 16, T], BF, kind="ExternalOutput").ap()

    with tile.TileContext(nc) as tc:
        with tc.tile_pool(name="wp", bufs=1) as wpool, \
             tc.tile_pool(name="xp", bufs=1) as xpool, \
             tc.tile_pool(name="yp", bufs=1) as ypool, \
             tc.tile_pool(name="st", bufs=7) as spool, \
             tc.tile_pool(name="lk", bufs=2) as lkpool, \
             tc.tile_pool(name="ps", bufs=PS_BUFS, space="PSUM") as ppool:

            wsb = wpool.tile([128, NWCOL], BF, tag="w")
            nc.scalar.dma_start(out=wsb[:], in_=w_d[:])
            bfs = wpool.tile([128, 8], F32, tag="bf")
            nc.scalar.activation(bfs[:, 0:6], wsb[:, NSLOT * 128: NSLOT * 128 + 6],
                                 mybir.ActivationFunctionType.Copy)

            def lhsT(slot, k=128, m=128, base=0):
                return wsb[base:base + k, slot * 128: slot * 128 + m]

            def bias(i):
                return bfs[:, i:i + 1]

            for _rep in range(reps):
                _emit_body(nc, tc, mybir, F32, F32R, BF, AF, OP, wsb, lhsT, bias,
                           xpool, ypool, spool, ppool, x_d, o_d, lkpool)
    nc.finalize()
    return nc


def _emit_body(nc, tc, mybir, F32, F32R, BF, AF, OP, wsb, lhsT, bias,
               xpool, ypool, spool, ppool, x_d, o_d, lkpool):
    if True:
        if True:
            # ---------------- input load: [120p = 4 x (29ch + ones), 2W]
            # host pre-pads x to [30, T+22] (10 zeros left, 12 right, ones row),
            # so each chunk load is one uniform DMA covering u in [6, Tc+28).
            X = xpool.tile([120, W2], BF, tag="x")
            NSEG = 2
            edges = [6 + (Tc + 22) * sg // NSEG for sg in range(NSEG + 1)]
            for sg in range(NSEG):
                lo, hi = edges[sg], edges[sg + 1]
                for b in range(BPC):
                    for c in range(4):
                        eng = nc.sync if (b * 4 + c) % 2 == 0 else nc.scalar
                        eng.dma_start(
                            out=X[30 * c:30 * c + 30, b * W + lo: b * W + hi],
                            in_=x_d[b, c, :, lo - 6: hi - 6])

            ST = lambda nm: spool.tile([128, W2], BF, tag="st", name=nm)  # noqa: E731

            def conv_pass(out_tile, rng, groups, evac):
                """groups: list of (out_p0, out_p1, taps); taps: (lhsT_ap, rhs_tile,
                rp0, rp1, delta).  Matmuls fill a 4-bank psum group in 512-col
                bank slices; one evacuation op drains the whole group."""
                for b in range(BPC):
                    for (glo, ghi) in _split(rng[0], rng[1], PS_GROUP):
                        gn = ghi - glo
                        ps = ppool.tile([128, PS_GROUP], F32, tag="ps", name="ps")
                        for (p0, p1, taps) in groups:
                            for (lo, hi) in _split(glo, ghi, 512):
                                n, off = hi - lo, lo - glo
                                for i, (lw, rt, rp0, rp1, d) in enumerate(taps):
                                    tp = (lw.base_partition(), p0) if p0 else None
                                    nc.tensor.matmul(
                                        ps[p0:p1, off:off + n], lw,
                                        rt[rp0:rp1, b * W + lo + d: b * W + hi + d],
                                        start=(i == 0), stop=(i == len(taps) - 1),
                                        tile_position=tp)
                        evac(ps[:, 0:gn], out_tile[:, b * W + glo: b * W + ghi])

            def conv_pass_pair(passes, rng):
                """Interleave the matmuls of two 64-contraction streams so they
                land on PE row tiles (0,0)/(64,0) and overlap 2x.  passes:
                list of (out_tile, taps, evac); taps as in conv_pass but with
                rhs base partitions 0:64 / 64:128 respectively."""
                for b in range(BPC):
                    for (glo, ghi) in _split(rng[0], rng[1], PS_GROUP):
                        gn = ghi - glo
                        pss = [ppool.tile([128, PS_GROUP], F32, tag="ps",
                                          name="psp%d" % j)
                               for j in range(len(passes))]
                        for (lo, hi) in _split(glo, ghi, 512):
                            n, off = hi - lo, lo - glo
                            ntap = max(len(t) for (_, t, _) in passes)
                            for i in range(ntap):
                                for j, (ot, taps, evac) in enumerate(passes):
                                    if i >= len(taps):
                                        continue
                                    lw, rt, rp0, rp1, d = taps[i]
                                    nc.tensor.matmul(
                                        pss[j][0:128, off:off + n], lw,
                                        rt[rp0:rp1, b * W + lo + d: b * W + hi + d],
                                        start=(i == 0), stop=(i == len(taps) - 1),
                                        tile_position=(rp0, 0))
                        for j, (ot, taps, evac) in enumerate(passes):
                            evac(pss[j][:, 0:gn], ot[:, b * W + glo: b * W + ghi])

            def act_evac(func, bias_ap, alpha, rnd=True):
                def f(ps, ot):
                    nc.scalar.activation(ot, ps, func, bias=bias_ap, scale=1.0,
                                         alpha=alpha)
                return f

            def dve_leaky(alpha, bias_ap=None):
                def f(ps, ot):
                    lk = lkpool.tile([128, PS_GROUP], BF, tag="lk", name="lk")
                    n = ps.shape[-1]
                    if bias_ap is None:
                        nc.vector.tensor_scalar(lk[:, 0:n], ps, alpha, None, OP.mult)
                        nc.vector.tensor_tensor(ot, ps, lk[:, 0:n],
                                                OP.max)
                    else:
                        # leaky(ps + b): lk = alpha*(ps+b); out = max(ps+b, lk)
                        nc.vector.tensor_scalar(lk[:, 0:n], ps, bias_ap, alpha,
                                                OP.add, OP.mult)
                        nc.vector.scalar_tensor_tensor(
                            ot, ps, bias_ap, lk[:, 0:n],
                            OP.add, OP.max)
                return f

            G1 = ST("G1")
            conv_pass(G1, (14, Tc + 34),
                      [(0, 128, [(lhsT(k, 120), X, 0, 120, k - 8) for k in range(3)])],
                      act_evac(AF.Prelu, 0.0, 0.02))
            E1 = ST("E1")
            conv_pass(E1, (13, Tc + 21),
                      [(0, 128, [(lhsT(k, 120), X, 0, 120, k - 8) for k in (1, 2)])],
                      dve_leaky(0.02))
            G2 = ST("G2")
            conv_pass(G2, (17, Tc + 33),
                      [(0, 128, [(lhsT(3 + k), G1, 0, 128, 2 * (k - 1)) for k in range(3)])],
                      act_evac(AF.Prelu, bias(0), 0.02))
            E2 = ST("E2")
            conv_pass(E2, (13, Tc + 21),
                      [(0, 128, [(lhsT(4), E1, 0, 128, 0), (lhsT(5), G1, 0, 128, 2)])],
                      dve_leaky(0.02, bias(0)))
            G3 = [ST("G3a"), ST("G3b")]
            conv_pass_pair(
                [(G3[p], [(lhsT(6 + k, 64, base=64 * p), G2,
                           64 * p, 64 * p + 64, 4 * (k - 1)) for k in range(3)],
                  act_evac(AF.Prelu, bias(1), 0.2)) for p in range(2)],
                (21, Tc + 29))
            E3 = [ST("E3a"), ST("E3b")]
            conv_pass_pair(
                [(E3[p], [(lhsT(7, 64, base=64 * p), E2, 64 * p, 64 * p + 64, 0),
                          (lhsT(8, 64, base=64 * p), G2, 64 * p, 64 * p + 64, 4)],
                  dve_leaky(0.2, bias(1))) for p in range(2)],
                (13, Tc + 21))
            H = [ST("Ha"), ST("Hb")]
            for p in range(2):
                conv_pass(H[p], (13, Tc + 21),
                          [(0, 128, [(lhsT(9), E3[p], 0, 128, 0),
                                     (lhsT(10), G3[p], 0, 128, 8)])],
                          act_evac(AF.Prelu, bias(2), 0.2))
            H1 = [ST("H1" + str(cidx)) for cidx in range(4)]
            for p in range(2):
                conv_pass_pair(
                    [(H1[2 * p + half],
                      [(lhsT(11, 64, base=64 * half), H[p],
                        64 * half, 64 * half + 64, 0)],
                      act_evac(AF.Prelu, bias(3), 0.02)) for half in range(2)],
                    (13, Tc + 21))
            A2 = [ST("A2a"), ST("A2b")]
            for p in range(2):
                conv_pass(A2[p], (13, Tc + 21),
                          [(0, 128, [(lhsT(12), H1[2 * p], 0, 128, 0),
                                     (lhsT(15), H1[2 * p + 1], 0, 128, 0)])],
                          act_evac(AF.Prelu, bias(4), 0.02))
            FFC = ST("FFC")
            conv_pass(FFC, (13, Tc + 21),
                      [(0, 128, [(lhsT(13), A2[0], 0, 128, 0),
                                 (lhsT(16), A2[1], 0, 128, 0)])],
                      act_evac(AF.Tanh, bias(5), 0.0, rnd=False))

            # ffc outside the valid t-range [0, T) must read as ZERO in the
            # window sum (reference zero-pads ffc, not just x).  The streams
            # compute phantom values there (biases propagate through zero
            # input), so zero them: chunk0 cols t=-3..-1, chunk3 t=T..T+4.
            for b in range(BPC):
                nc.gpsimd.memset(FFC[0:32, b * W + 13: b * W + 16], 0.0)
                nc.gpsimd.memset(FFC[96:128, b * W + Tc + 16: b * W + Tc + 21], 0.0)

            # ---------------- window sum (DVE tree): ws[t] = sum_{d=-3..4} ffc[t+d]
            S1 = ST("S1")
            for b in range(BPC):
                o = b * W
                nc.vector.tensor_tensor(S1[:, o + 13: o + Tc + 19],
                                        FFC[:, o + 13: o + Tc + 19],
                                        FFC[:, o + 14: o + Tc + 20], OP.add)
            for b in range(BPC):
                o = b * W
                nc.vector.tensor_tensor(FFC[:, o + 13: o + Tc + 17],
                                        S1[:, o + 13: o + Tc + 17],
                                        S1[:, o + 15: o + Tc + 19], OP.add)
            for b in range(BPC):
                o = b * W
                nc.vector.tensor_tensor(S1[:, o + 16: o + Tc + 16],
                                        FFC[:, o + 13: o + Tc + 13],
                                        FFC[:, o + 17: o + Tc + 17], OP.add)

            # ---------------- final fc + clip -> Y [64 = 4ch x 16cls, BPC*Tc]
            Y = ypool.tile([64, BPC * Tc], BF, tag="y")
            for b in range(BPC):
                for (glo, ghi) in _split(16, Tc + 16, PS_GROUP):
                    ps = ppool.tile([128, PS_GROUP], F32, tag="ps", name="ps")
                    for (lo, hi) in _split(glo, ghi, 512):
                        n, off = hi - lo, lo - glo
                        nc.tensor.matmul(ps[0:64, off:off + n], lhsT(14, 128, 64),
                                         S1[:, b * W + lo: b * W + hi],
                                         start=True, stop=True)
                    nc.vector.tensor_scalar(
                        Y[:, b * Tc + glo - 16: b * Tc + ghi - 16],
                        ps[0:64, 0:ghi - glo], 0.0, 1.0, OP.max, OP.min)

            # ---------------- output DMA: Y rows 16c..16c+16 -> out[b, :, c*Tc:...]
            for b in range(BPC):
                for half in range(2):
                    h0, h1 = half * (Tc // 2), (half + 1) * (Tc // 2)
                    for c4 in range(4):
                        eng = nc.sync if c4 % 2 == 0 else nc.scalar
                        eng.dma_start(
                            out=o_d[b, :, c4 * Tc + h0: c4 * Tc + h1],
                            in_=Y[16 * c4:16 * c4 + 16, b * Tc + h0: b * Tc + h1])


def _get_program(reps=1):
    global _PROG
    if _PROG is None:
        _PROG = {}
    if reps not in _PROG:
        _PROG[reps] = _build_program(reps)
    return _PROG[reps]


def kernel(**inputs):
    from concourse.bass_utils import run_bass_kernel_spmd

    x = np.asarray(inputs['speech_features'], np.float32)
    xa = np.zeros((B, C + 1, T + 22), np.float32)
    xa[:, :C, 10:10 + T] = x
    xa[:, C, :] = 1.0
    xc = np.empty((B, 4, C + 1, Tc + 22), np.float32)
    for c in range(4):
        xc[:, c] = xa[:, :, c * Tc: c * Tc + Tc + 22]
    xa = xc.astype(BF16)
    wp = _pack_weights({k: np.asarray(v, np.float32) for k, v in inputs.items()
                        if k != 'speech_features'}).astype(BF16)
    nc = _get_program()
    in_maps = [{"x": xa[i * BPC:(i + 1) * BPC], "wpack": wp} for i in range(NCORES)]
    res = run_bass_kernel_spmd(nc, in_maps, core_ids=list(range(NCORES)))
    outs = [r["out"].transpose(0, 2, 1) for r in res.results]
    return np.ascontiguousarray(np.concatenate(outs, axis=0).astype(np.float32))

